# revision 1
# baseline (speedup 1.0000x reference)
"""Trainium2 Bass kernel for nn_ConsistencyLoss.

Strategy (pure data-parallel over the agent dim N, 8 cores):
  - Host pads N 20000 -> 20480, builds agent-major trajectory tensors
    lp/ln = pred[..., :2] (agent-major) + pad offsets, and shards 2560
    agents per core.
  - On-device per core: endpoint distances -> 36-entry dist matrix per
    agent -> match scores over all 720 mode permutations via a PE matmul
    with a constant selection matrix -> argmin via Max8/MaxIndex (on
    negated scores) -> permutation one-hot (PE) -> selected trajectories
    (mask-multiply + tree sum) -> smooth-L1 sums, masked by validity.
  - Each core returns 3 partial sums (cons, reg, n_valid); host combines.

Self-contained: hardcodes shapes/sharding; only needs /opt/trn_rl_repo.
"""

import sys
from itertools import permutations

import numpy as np

if "/opt/trn_rl_repo" not in sys.path:
    sys.path.insert(0, "/opt/trn_rl_repo")

NUM_MODES = 6
T = 30
NPERM = 720
N_FULL = 20000
N_CORES = 8
PPART = 128

PERMS = np.array(list(permutations(range(NUM_MODES))), dtype=np.int32)  # [720, 6]


CONST_COLS = 720 + 216 + 6 + 128 + 120  # 1190


def _host_consts():
    """One packed [128, 1190] constant block: negS | stt | iota2 | ident | ones120."""
    S = np.zeros((36, NPERM), np.float32)
    for p in range(NPERM):
        for i in range(NUM_MODES):
            S[i * 6 + PERMS[p, i], p] = 1.0
    stt = S.T.reshape(6, 120, 36).transpose(1, 0, 2)      # [120, 6, 36]
    iota2 = (np.arange(6)[None, :] * 120 + np.arange(120)[:, None]).astype(
        np.float32
    )                                                     # [120, 6]
    c = np.zeros((PPART, CONST_COLS), np.float32)
    c[0:36, 0:720] = -S
    c[0:120, 720:936] = stt.reshape(120, 216)
    c[0:120, 936:942] = iota2
    c[:, 942:1070] = np.eye(PPART, dtype=np.float32)
    c[0:1, 1070:1190] = 1.0
    c[:, 1070] = 1.0  # full-partition ones column (ones1/onebias)
    return c


def build_nc(nsh):
    """Build the per-core Bass program for a shard of `nsh` agents."""
    import concourse.bass as bass
    import concourse.bacc as bacc
    import concourse.mybir as mybir
    import concourse.tile as tile

    f32 = mybir.dt.float32
    u32 = mybir.dt.uint32
    Alu = mybir.AluOpType
    Act = mybir.ActivationFunctionType
    AX = mybir.AxisListType

    A = nsh // PPART  # agents per partition == number of blocks
    assert A * PPART == nsh

    nc = bacc.Bacc(None, target_bir_lowering=False, debug=False)

    lp_d = nc.declare_dram_parameter("lp", [nsh, NUM_MODES, T, 2], f32, False)
    ln_d = nc.declare_dram_parameter("ln", [nsh, NUM_MODES, T, 2], f32, False)
    sm_d = nc.declare_dram_parameter("smalls", [nsh, 15], f32, False)
    hd_d = nc.declare_dram_parameter("heads", [nsh, NUM_MODES, 4], f32, False)
    cst_d = nc.declare_dram_parameter("consts", [PPART, CONST_COLS], f32, False)
    out_d = nc.declare_dram_parameter("partials", [3, 1], f32, True)

    from concourse.tile_rust import add_dep_helper

    with tile.TileContext(nc) as tc:
        with (
            tc.tile_pool(name="big", bufs=4) as big,
            tc.tile_pool(name="mid", bufs=1) as mid,
            tc.tile_pool(name="sml", bufs=2) as sml,
            tc.tile_pool(name="cst", bufs=1) as cst,
            tc.tile_pool(name="pnm", bufs=1, space="PSUM") as pnm,
            tc.tile_pool(name="ptd", bufs=2, space="PSUM") as ptd,
            tc.tile_pool(name="pw", bufs=2, space="PSUM") as pw,
            tc.tile_pool(name="psm", bufs=1, space="PSUM") as psm,
        ):
            # ---- packed constant / small-input tiles ----
            cstt = cst.tile([PPART, CONST_COLS], f32)
            dma_insts = []
            dma_insts.append(nc.sync.dma_start(cstt[:], cst_d[:]))
            negs = cstt[0:36, 0:720]
            stt = cstt[0:120, 720:936].rearrange("p (c w) -> p c w", c=6)
            iota2 = cstt[0:120, 936:942]
            ident = cstt[:, 942:1070]
            ones120 = cstt[0:1, 1070:1190]
            ones1 = cstt[:, 1070:1071]

            smt = cst.tile([PPART, A, 15], f32)
            dma_insts.append(
                nc.sync.dma_start(
                    smt[:], sm_d[:].rearrange("(p a) x -> p (a x)", p=PPART)
                )
            )
            pad = smt[:, :, 0:12].rearrange("p a (f c) -> p a f c", f=NUM_MODES)
            tg = smt[:, :, 12:14]
            val = smt[:, :, 14:15].rearrange("p a x -> p (a x)")

            heads = cst.tile([PPART, A, NUM_MODES, 4], f32)
            dma_insts.append(
                nc.sync.dma_start(
                    heads[:], hd_d[:].rearrange("(p a) f x -> p (a f x)", p=PPART)
                )
            )

            # ---- bulk trajectory loads (agent-major) ----
            ln = big.tile([PPART, A, NUM_MODES, T * 2], f32, tag="big")
            dma_insts.append(
                nc.sync.dma_start(
                    ln[:], ln_d[:].rearrange("(p a) f t c -> p (a f t c)", p=PPART)
                )
            )
            lp = big.tile([PPART, A, NUM_MODES, T * 2], f32, tag="big")
            dma_insts.append(
                nc.sync.dma_start(
                    lp[:], lp_d[:].rearrange("(p a) f t c -> p (a f t c)", p=PPART)
                )
            )

            # ---- endpoint distance matrix: dist [128, A, 6, 6] ----
            # ISA APs allow at most 3 free dims, so handle x/y separately.
            ddx = mid.tile([PPART, A, 6, 6], f32, tag="ddx")
            ddy = mid.tile([PPART, A, 6, 6], f32, tag="ddy")
            for c, ddc in ((0, ddx), (1, ddy)):
                lpe = heads[:, :, :, c : c + 1]     # [128, A, 6, 1]
                lne = heads[:, :, :, 2 + c : 3 + c]
                nc.vector.tensor_sub(
                    ddc[:],
                    lpe.rearrange("p a i x -> p a (i x)")
                    .unsqueeze(3)
                    .broadcast_to([PPART, A, 6, 6]),
                    lne.rearrange("p a j x -> p a (j x)")
                    .unsqueeze(2)
                    .broadcast_to([PPART, A, 6, 6]),
                )
            sqx = mid.tile([PPART, A, 6, 6], f32, tag="sqx")
            nc.scalar.activation(sqx[:], ddx[:], Act.Square, bias=0.0)
            sqy = mid.tile([PPART, A, 6, 6], f32, tag="sqy")
            nc.scalar.activation(sqy[:], ddy[:], Act.Square, bias=0.0)
            dsum = mid.tile([PPART, A, 6, 6], f32, tag="ds")
            nc.vector.tensor_add(dsum[:], sqx[:], sqy[:])
            dist = mid.tile([PPART, A, 6, 6], f32, tag="di")
            nc.scalar.activation(dist[:], dsum[:], Act.Sqrt, bias=0.0)

            # ---- per-block: match -> argmin -> one-hot W ----
            w_all = cst.tile([PPART, A, 40], mybir.dt.uint8)
            sel = big.tile([PPART, A, NUM_MODES, T * 2], f32, tag="big")

            def match_block(a):
                td = ptd.tile([36, PPART], f32, tag="td")
                nc.tensor.transpose(td[:], dist[:, a, :, :], ident)
                tds = sml.tile([36, PPART], f32, tag="tds")
                nc.vector.tensor_copy(tds[:], td[:])

                nm = pnm.tile([PPART, NPERM], f32, tag="nm")
                nc.tensor.matmul(nm[:, 0:512], tds[:], negs[:, 0:512])
                nc.tensor.matmul(nm[:, 512:NPERM], tds[:], negs[:, 512:NPERM])
                nms = sml.tile([PPART, NPERM], f32, tag="nms")
                nc.scalar.copy(nms[:], nm[:])

                m8 = sml.tile([PPART, 8], f32, tag="m8")
                nc.vector.max(m8[:], nms[:])
                # permutation one-hot: match-score equality against the max
                # (exact f32 compare: the max is one of the row's elements)
                e = sml.tile([PPART, NPERM], f32, tag="e")
                nc.vector.tensor_scalar(
                    e[:], nms[:], m8[:, 0:1], None, Alu.is_equal
                )
                etp = psm.tile([120, 6, PPART], f32, tag="sm")
                for c in range(6):
                    nc.tensor.transpose(
                        etp[:, c, :], e[:, 120 * c : 120 * (c + 1)], ident
                    )
                ets = sml.tile([120, 6, PPART], f32, tag="ets")
                nc.scalar.copy(ets[:], etp[:])
                wp = pw.tile([PPART, 36], f32, tag="w")
                for c in range(6):
                    nc.tensor.matmul(
                        wp[:],
                        ets[:, c, :],
                        stt[:, c, :],
                        start=(c == 0),
                        stop=(c == 5),
                    )
                nc.vector.tensor_copy(w_all[:, a, 0:36], wp[:])

            def sel_half(a0, a1):
                aw = a1 - a0
                for i in range(NUM_MODES):
                    for j in range(NUM_MODES):
                        mask_ij = w_all[
                            :, a0:a1, i * 6 + j : i * 6 + j + 1
                        ].broadcast_to([PPART, aw, T * 2])
                        data_j = ln[:, a0:a1, j : j + 1, :].rearrange(
                            "p a x tc -> p a (x tc)"
                        )
                        out_i = sel[:, a0:a1, i : i + 1, :].rearrange(
                            "p a x tc -> p a (x tc)"
                        )
                        nc.vector.copy_predicated(out_i, mask_ij, data_j)

            # ---- selected trajectories via predicated copies ----
            # Every (agent, mode-i) row has exactly one hot j in W, so six
            # masked overwrites fully populate sel. Done in agent-halves so
            # the first half's selection overlaps the second half of the
            # per-block match loop (DVE fills ACT/PE bubbles).
            half = A // 2 if A >= 2 else A
            for a in range(half):
                match_block(a)
            sel_half(0, half)
            for a in range(half, A):
                match_block(a)
            sel_half(half, A)

            # ---- smooth-L1 sums on selected pairs ----
            d = big.tile([PPART, A, NUM_MODES, T * 2], f32, tag="big")
            nc.vector.tensor_sub(d[:], lp[:], sel[:])
            sa = sml.tile([PPART, A], f32, tag="sa")
            nc.vector.tensor_reduce(
                sa[:],
                d[:].rearrange("p a f tc -> p a (f tc)"),
                axis=AX.X,
                op=Alu.add,
                apply_absolute_value=True,
            )
            ab = big.tile([PPART, A, NUM_MODES, T * 2], f32, tag="big")
            nc.scalar.activation(ab[:], d[:], Act.Abs, bias=0.0)
            m = big.tile([PPART, A, NUM_MODES, T * 2], f32, tag="big")
            nc.vector.tensor_scalar_min(m[:], ab[:], 1.0)
            r2 = big.tile([PPART, A, NUM_MODES, T * 2], f32, tag="big")
            nc.scalar.activation(r2[:], m[:], Act.Square, bias=1.0, scale=-1.0)
            sr = sml.tile([PPART, A], f32, tag="sr")
            nc.vector.tensor_reduce(
                sr[:],
                r2[:].rearrange("p a f tc -> p a (f tc)"),
                axis=AX.X,
                op=Alu.add,
            )
            ca = sml.tile([PPART, A], f32, tag="ca")
            nc.vector.tensor_scalar(ca[:], sr[:], 0.5, -0.5 * 360.0, Alu.mult, Alu.add)
            cb = sml.tile([PPART, A], f32, tag="cb")
            nc.vector.tensor_add(cb[:], ca[:], sa[:])

            # ---- reg loss terms ----
            rd = sml.tile([PPART, A, NUM_MODES, 2], f32, tag="rd")
            nc.vector.tensor_sub(
                rd[:], pad, tg.unsqueeze(2).broadcast_to([PPART, A, 6, 2])
            )
            ra = sml.tile([PPART, A], f32, tag="ra")
            nc.vector.tensor_reduce(
                ra[:],
                rd[:].rearrange("p a f c -> p a (f c)"),
                axis=AX.X,
                op=Alu.add,
                apply_absolute_value=True,
            )
            rab = sml.tile([PPART, A, NUM_MODES, 2], f32, tag="rab")
            nc.scalar.activation(rab[:], rd[:], Act.Abs, bias=0.0)
            rm = sml.tile([PPART, A, NUM_MODES, 2], f32, tag="rm")
            nc.vector.tensor_scalar_min(rm[:], rab[:], 1.0)
            rr2 = sml.tile([PPART, A, NUM_MODES, 2], f32, tag="rr2")
            nc.scalar.activation(rr2[:], rm[:], Act.Square, bias=1.0, scale=-1.0)
            rr = sml.tile([PPART, A], f32, tag="rr")
            nc.vector.tensor_reduce(
                rr[:],
                rr2[:].rearrange("p a f c -> p a (f c)"),
                axis=AX.X,
                op=Alu.add,
            )
            rca = sml.tile([PPART, A], f32, tag="rca")
            nc.vector.tensor_scalar(rca[:], rr[:], 0.5, -0.5 * 12.0, Alu.mult, Alu.add)
            rcb = sml.tile([PPART, A], f32, tag="rcb")
            nc.vector.tensor_add(rcb[:], rca[:], ra[:])

            # ---- masked partition sums -> 3 partials ----
            cv = sml.tile([PPART, A], f32, tag="cv")
            nc.vector.tensor_mul(cv[:], cb[:], val)
            rv = sml.tile([PPART, A], f32, tag="rv")
            nc.vector.tensor_mul(rv[:], rcb[:], val)
            acc = sml.tile([PPART, 3], f32, tag="acc3")
            nc.vector.tensor_reduce(acc[:, 0:1], cv[:], axis=AX.X, op=Alu.add)
            nc.vector.tensor_reduce(acc[:, 1:2], rv[:], axis=AX.X, op=Alu.add)
            nc.vector.tensor_reduce(acc[:, 2:3], val, axis=AX.X, op=Alu.add)

            fp = ptd.tile([3, 1], f32, tag="td")
            nc.tensor.matmul(fp[:], acc[:], ones1)
            fps = sml.tile([3, 1], f32, tag="fps")
            nc.scalar.copy(fps[:], fp[:])
            nc.sync.dma_start(out_d[:], fps[:])

    nc.finalize()
    return nc


def _prep_host(pred_past, pred_now, pad_loc, pad_loc_mask, pad_loc_target, n_pad):
    """Build padded agent-major host tensors."""
    n = pred_past.shape[1]
    lp = np.zeros((n_pad, NUM_MODES, T, 2), np.float32)
    ln = np.zeros((n_pad, NUM_MODES, T, 2), np.float32)
    smalls = np.zeros((n_pad, 15), np.float32)

    pp = np.ascontiguousarray(pred_past[..., :2].transpose(1, 0, 2, 3))
    pn = np.ascontiguousarray(pred_now[..., :2].transpose(1, 0, 2, 3))
    pl = np.ascontiguousarray(pad_loc.transpose(1, 0, 2))
    lp[:n] = pp + pl[:, :, None, :]
    ln[:n] = pn + pad_loc_target[:, None, None, :]
    smalls[:n, 0:12] = pl.reshape(n, 12)
    smalls[:n, 12:14] = pad_loc_target
    smalls[:n, 14] = (~pad_loc_mask).astype(np.float32)
    heads = np.zeros((n_pad, NUM_MODES, 4), np.float32)
    heads[:, :, 0:2] = lp[:, :, T - 1, :]
    heads[:, :, 2:4] = ln[:, :, T - 1, :]
    return lp, ln, smalls, heads


_CACHE = {}
LAST_RESULT = None


def kernel(pred_past, pred_now, pad_loc, pad_loc_mask, pad_loc_target):
    global LAST_RESULT
    from concourse.bass_utils import run_bass_kernel_spmd

    pred_past = np.asarray(pred_past, np.float32)
    pred_now = np.asarray(pred_now, np.float32)
    pad_loc = np.asarray(pad_loc, np.float32)
    pad_loc_mask = np.asarray(pad_loc_mask, bool)
    pad_loc_target = np.asarray(pad_loc_target, np.float32)

    n = pred_past.shape[1]
    n_pad = ((n + N_CORES * PPART - 1) // (N_CORES * PPART)) * (N_CORES * PPART)
    nsh = n_pad // N_CORES

    lp, ln, smalls, heads = _prep_host(
        pred_past, pred_now, pad_loc, pad_loc_mask, pad_loc_target, n_pad
    )
    consts = _host_consts()

    if nsh not in _CACHE:
        _CACHE[nsh] = build_nc(nsh)
    nc = _CACHE[nsh]

    in_maps = []
    for c in range(N_CORES):
        s = slice(c * nsh, (c + 1) * nsh)
        in_maps.append(
            {
                "lp": lp[s],
                "ln": ln[s],
                "smalls": smalls[s],
                "heads": heads[s],
                "consts": consts,
            }
        )

    res = run_bass_kernel_spmd(nc, in_maps, list(range(N_CORES)))
    LAST_RESULT = res
    parts = np.stack([r["partials"][:, 0] for r in res.results])  # [8, 3]
    c_sum = parts[:, 0].sum()
    r_sum = parts[:, 1].sum()
    n_valid = max(parts[:, 2].sum(), 1.0)
    reg_loss = np.float32(r_sum / (NUM_MODES * 2 * n_valid))
    cons_loss = np.float32(c_sum / (NUM_MODES * T * 2 * n_valid))
    return (reg_loss, cons_loss)

